# revision 11
# baseline (speedup 1.0000x reference)
"""MoE top-2 routing kernel for 8 Trainium2 NeuronCores.

Strategy (expert-parallel, per the sharding hint):
  - Host computes the (tiny) router in float64: logits -> softmax -> top-2 ->
    renormalize.  67 MFLOP total, ~0.05% of the model FLOPs.  Selection was
    verified tie-safe: min prob gap between 2nd/3rd expert is ~8e-6 while
    cross-backend fp32 logit noise is ~3e-7.
  - Tokens are dispatched (gathered) per expert on the host; expert e's token
    batch, padded to capacity C, goes to core e along with expert e's weights
    (pre-transposed on host into the exact SBUF-friendly layouts the kernel
    consumes, cast to bf16).
  - Each core runs a dense FFN over its C tokens:
        hT = gelu(w1T.T-contractions)   (PSUM fp32 accum, bias fused in ACT)
        oT = w2-contractions over hT
    with D/H features on the partition axis end-to-end, so no on-device
    transposes are needed anywhere.
  - Host applies the combine weights and scatter-adds per-expert outputs back
    into the full [B,S,D] output (each token appears in exactly 2 experts).

v3 schedule notes (from trace analysis):
  - The ramp is HBM-bandwidth-bound: the PE cannot start until xb0+w1[0]
    arrive (~2.6us of transfer after the ~8.3us fixed prologue), and the
    first `lead` groups consume slab bandwidth at ~200GB/s.  v1 lost 8.3us
    because xblk2 and a hoisted 1MB w2[0] fetch jumped ahead of w1 slabs in
    the queues.  Queue FIFO order is now explicit: xb0 halves first (one per
    queue), then w1 slabs alternating, with xblk1/xblk2 inserted after
    w1[4]/w1[5].
  - w2 fetches are hard-gated behind layer-1 progress: the scheduler hoists
    any ungated DMA descriptor to the top of the stream, so each early w2
    tile's buffer is first touched by a tiny scalar memset placed (in scalar
    program order) after a mid-L1 activation; the WAR dependency delays the
    fetch until ~30us, long before L2 needs it at ~125us.
  - C pads to /8 (1072 here) instead of /32 (1088): matmul column counts and
    DMA row lengths have no /32 requirement.  Saves 3.4us of matmul time.
  - 2 DMA queues only (sync+scalar); fewer queues/semaphores also trim the
    fixed teardown epilogue.

Per-core layouts (E=8 experts, D=1024, H=4096, C tokens):
  xT  [128, 8*C]         bf16   block-major concat of xT[p, ko, t] = x_g[t, ko*128+p]
  w1t [128, 32, 8, 128]  bf16   w1t[p, ho, ko, j]  = w1[e][ho*128+j, ko*128+p]
  w2t [128, 8, 32, 128]  bf16   w2t[p, do, ko, j]  = w2[e][do*128+j, ko*128+p]
  b1t [128, 32]          f32    b1t[p, ho]         = b1[e][ho*128+p]
  oT  [128, 8, C]        f32    oT[p, do, t]       = o_g[t, do*128+p]
"""

import numpy as np
import ml_dtypes

TOP_K = 2
P = 128
D = 1024
H = 4096
E = 8

_COMPILED = {}  # C -> compiled Bacc instance


def _token_blocks(C):
    """First block 320 (fast start; slab demand still below delivery),
    remaining blocks <=512, all >=232 so LDWEIGHTS stays hidden."""
    if C <= 512:
        return [C]
    b0 = 320
    rem = C - b0
    nblk = -(-rem // 480)
    sizes = [b0]
    for i in range(nblk):
        s = -(-rem // (nblk - i))
        s = ((s + 7) // 8) * 8
        s = min(s, rem)
        sizes.append(s)
        rem -= s
    assert sum(sizes) == C and all(s <= 512 for s in sizes), sizes
    return sizes


def _build_ffn_kernel(C):
    import concourse.mybir as mybir
    import concourse.tile as tile
    from concourse import bacc

    blocks = _token_blocks(C)
    starts = [sum(blocks[:i]) for i in range(len(blocks))]
    NTMAX = max(blocks)
    nb = len(blocks)
    bf16 = mybir.dt.bfloat16
    f32 = mybir.dt.float32

    nc = bacc.Bacc("TRN2", target_bir_lowering=False, debug=False)
    xT = nc.dram_tensor("xT", [P, D // P * C], bf16, kind="ExternalInput").ap()
    w1t = nc.dram_tensor("w1t", [P, H // P, D // P, P], bf16, kind="ExternalInput").ap()
    w2t = nc.dram_tensor("w2t", [P, D // P, H // P, P], bf16, kind="ExternalInput").ap()
    b1t = nc.dram_tensor("b1t", [P, H // P], f32, kind="ExternalInput").ap()
    oT = nc.dram_tensor("oT", [P, D // P, C], f32, kind="ExternalOutput").ap()

    KH = D // P // 2  # 4: ko half-count for the split first block / slab

    with tile.TileContext(nc) as tc:
        with (
            tc.tile_pool(name="const", bufs=1) as cpool,
            tc.tile_pool(name="resident", bufs=1) as rpool,
            tc.tile_pool(name="warm", bufs=1) as warmpool,
            tc.tile_pool(name="w1p", bufs=6) as w1pool,
            tc.tile_pool(name="w2p", bufs=3) as w2pool,
            tc.tile_pool(name="ost", bufs=4) as opool,
            tc.tile_pool(name="ps", bufs=4, space="PSUM") as pspool,
            tc.tile_pool(name="wps", bufs=1, space="PSUM") as wpspool,
        ):
            # PE warm-up: dependency-free matmuls keep the HAM clock-gate at
            # 8/8 while the first input DMAs are in flight.
            wsrc = warmpool.tile([P, 512], bf16)
            nc.gpsimd.memset(wsrc[:], 0.0)
            wps = wpspool.tile([P, 512], f32)
            for _ in range(11):
                nc.tensor.matmul(wps[:], wsrc[:, :P], wsrc[:], start=True, stop=True)

            # ---- Input DMA schedule (FIFO order per queue IS the schedule):
            # qA (sync):   w1 pairs (0,1), (2,3), ... — a pure slab stream
            #              with 4096B rows (2048B rows proved ~2-3x slower
            #              per queue in the trace).
            # qB (scalar): xb0, xblk1, xblk2 (big rows), then gated w2.
            # qC (gpsimd): b1 (tiny).
            sz0 = blocks[0]
            b1_sb = cpool.tile([P, H // P], f32)
            nc.gpsimd.dma_start(b1_sb[:], b1t[:])

            w1_tiles = {}

            def fetch_w1(ho):
                # fetch the aligned pair (ho&~1, ho|1) as one 4096B-row DMA
                base = ho & ~1
                t = w1pool.tile([P, 2, D // P, P], bf16, tag="w1s")
                nc.sync.dma_start(t[:], w1t[:, base : base + 2])
                w1_tiles[base] = t
                w1_tiles[base + 1] = t

            fetch_w1(0)
            x_blks = []
            for blk in range(nb):
                st, sz = starts[blk], blocks[blk]
                xb = rpool.tile([P, D // P * sz], bf16, tag=f"xb{blk}")
                nc.scalar.dma_start(xb[:], xT[:, D // P * st : D // P * (st + sz)])
                x_blks.append(xb)

            h_sb = rpool.tile([P, H // P, C], bf16)

            def w1_src(ho, ko):
                return w1_tiles[ho][:, ho % 2, ko, :]

            def x_src(blk, ko):
                sz = blocks[blk]
                return x_blks[blk][:, ko * sz : (ko + 1) * sz]

            # Group order: lead = first 6 ho rows on block 0 (x blocks 1/2
            # still in flight), then their remaining blocks, then ho-major
            # triples.  Keeps slab lifetimes short (pool of 10 suffices) and
            # slab demand low after the ramp.
            lead = min(6, H // P) if nb >= 2 else 0
            pairs = [(k, 0) for k in range(lead)]
            for k in range(lead):
                pairs += [(k, b) for b in range(1, nb)]
            for ho in range(lead, H // P):
                pairs += [(ho, b) for b in range(nb)]

            # Layer 1: hT[:, ho, t] = gelu(sum_ko w1t[:,ho,ko,:].T @ x + b1)
            w2_early = []
            for pi, (ho, blk) in enumerate(pairs):
                if ho not in w1_tiles:
                    fetch_w1(ho)
                st, sz = starts[blk], blocks[blk]
                ps = pspool.tile([P, NTMAX], f32, tag="ps")
                for ko in range(D // P):
                    nc.tensor.matmul(
                        ps[:, :sz],
                        w1_src(ho, ko),
                        x_src(blk, ko),
                        start=(ko == 0),
                        stop=(ko == D // P - 1),
                    )
                nc.scalar.activation(
                    h_sb[:, ho, st : st + sz],
                    ps[:, :sz],
                    mybir.ActivationFunctionType.Gelu,
                    bias=b1_sb[:, ho : ho + 1],
                )
                if pi == 16:
                    # Early w2 fetches, gated so they cannot hoist ahead of
                    # the ramp: each buffer is touched by a scalar memset
                    # that (scalar is in-order) runs after the ACT above.
                    for do in range(min(3, D // P)):
                        w2s = w2pool.tile([P, H // P, P], bf16, tag="w2s")
                        nc.scalar.memzero(w2s[:, 0:2])
                        eng = nc.sync if do % 2 == 0 else nc.scalar
                        eng.dma_start(w2s[:], w2t[:, do])
                        w2_early.append(w2s)

            # Layer 2: oT[:, do, t] = sum_ko w2t[:,do,ko,:].T @ hT[:,ko,t]
            for do in range(D // P):
                if do < len(w2_early):
                    w2s = w2_early[do]
                else:
                    w2s = w2pool.tile([P, H // P, P], bf16, tag="w2s")
                    eng = nc.sync if do % 2 == 0 else nc.scalar
                    eng.dma_start(w2s[:], w2t[:, do])
                for bi, (st, sz) in enumerate(zip(starts, blocks)):
                    ps = pspool.tile([P, NTMAX], f32, tag="ps")
                    for ko in range(H // P):
                        nc.tensor.matmul(
                            ps[:, :sz],
                            w2s[:, ko, :],
                            h_sb[:, ko, st : st + sz],
                            start=(ko == 0),
                            stop=(ko == H // P - 1),
                        )
                    last = do == D // P - 1 and bi == nb - 1
                    if not last:
                        ob = opool.tile([P, NTMAX], f32, tag="ob")
                        nc.vector.tensor_copy(ob[:, :sz], ps[:, :sz])
                        nc.scalar.dma_start(oT[:, do, st : st + sz], ob[:, :sz])
                    else:
                        # Final eviction is on the critical path: split it so
                        # the first half's DMA overlaps the second half's
                        # copy, using both queues.
                        hsz = sz // 2
                        ob = opool.tile([P, NTMAX], f32, tag="ob")
                        nc.vector.tensor_copy(ob[:, :hsz], ps[:, :hsz])
                        nc.sync.dma_start(oT[:, do, st : st + hsz], ob[:, :hsz])
                        nc.vector.tensor_copy(ob[:, hsz:sz], ps[:, hsz:sz])
                        nc.scalar.dma_start(
                            oT[:, do, st + hsz : st + sz], ob[:, hsz:sz]
                        )

    nc.compile()
    return nc


def _route_host(x_flat, router_w):
    """Float64 router: returns per-expert (token_idx, combine_weight)."""
    logits = x_flat.astype(np.float64) @ router_w.astype(np.float64).T
    m = logits.max(axis=-1, keepdims=True)
    p = np.exp(logits - m)
    p /= p.sum(axis=-1, keepdims=True)
    order = np.argsort(-p, axis=-1)
    topi = order[:, :TOP_K]
    topw = np.take_along_axis(p, topi, axis=-1)
    topw /= topw.sum(axis=-1, keepdims=True)

    idx_list, wgt_list = [], []
    for e in range(E):
        mask = topi == e  # [T, TOP_K]; at most one True per row
        rows = np.nonzero(mask.any(axis=-1))[0]
        w = topw[rows][mask[rows]]
        idx_list.append(rows)
        wgt_list.append(w.astype(np.float32))
    return idx_list, wgt_list


def kernel(x, router_w, w1, b1, w2, b2):
    from concourse import bass_utils

    x = np.asarray(x)
    router_w = np.asarray(router_w)
    w1 = np.asarray(w1)
    b1 = np.asarray(b1)
    w2 = np.asarray(w2)
    b2 = np.asarray(b2)

    B, S, _ = x.shape
    T = B * S
    x_flat = x.reshape(T, D)

    idx_list, wgt_list = _route_host(x_flat, router_w)
    max_cnt = max(len(i) for i in idx_list)
    C = ((max_cnt + 7) // 8) * 8

    if C not in _COMPILED:
        _COMPILED[C] = _build_ffn_kernel(C)
    nc = _COMPILED[C]

    blocks = _token_blocks(C)
    starts = [sum(blocks[:i]) for i in range(len(blocks))]
    bf = ml_dtypes.bfloat16
    in_maps = []
    for e in range(E):
        idx = idx_list[e]
        n_e = len(idx)
        # xT [128, 8*C] block-major: pad tokens to C with zeros
        xg = np.zeros((C, D), np.float32)
        xg[:n_e] = x_flat[idx]
        xT_full = xg.T.reshape(D // P, P, C).transpose(1, 0, 2)  # [128, 8, C]
        xT_d = np.concatenate(
            [xT_full[:, :, st : st + sz].reshape(P, -1) for st, sz in zip(starts, blocks)],
            axis=1,
        ).astype(bf)
        w1_d = np.ascontiguousarray(
            w1[e].reshape(H // P, P, D // P, P).transpose(3, 0, 2, 1)
        ).astype(bf)
        w2_d = np.ascontiguousarray(
            w2[e].reshape(D // P, P, H // P, P).transpose(3, 0, 2, 1)
        ).astype(bf)
        b1_d = np.ascontiguousarray(b1[e].reshape(H // P, P).T).astype(np.float32)
        in_maps.append({"xT": xT_d, "w1t": w1_d, "w2t": w2_d, "b1t": b1_d})

    res = bass_utils.run_bass_kernel_spmd(nc, in_maps, core_ids=list(range(E)))

    out = np.zeros((T, D), np.float32)
    for e in range(E):
        idx = idx_list[e]
        n_e = len(idx)
        oT = res.results[e]["oT"]  # [128, 8, C]
        o_g = oT.transpose(1, 0, 2).reshape(D, C)[:, :n_e].T  # [n_e, D]
        out[idx] += wgt_list[e][:, None] * (o_g + b2[e][None, :])
    return out.reshape(B, S, D).astype(np.float32)


# revision 12
# speedup vs baseline: 1.0100x; 1.0100x over previous
"""MoE top-2 routing kernel for 8 Trainium2 NeuronCores.

Strategy (expert-parallel, per the sharding hint):
  - Host computes the (tiny) router in float64: logits -> softmax -> top-2 ->
    renormalize.  67 MFLOP total, ~0.05% of the model FLOPs.  Selection was
    verified tie-safe: min prob gap between 2nd/3rd expert is ~8e-6 while
    cross-backend fp32 logit noise is ~3e-7.
  - Tokens are dispatched (gathered) per expert on the host; expert e's token
    batch, padded to capacity C, goes to core e along with expert e's weights
    (pre-transposed on host into the exact SBUF-friendly layouts the kernel
    consumes, cast to bf16).
  - Each core runs a dense FFN over its C tokens:
        hT = gelu(w1T.T-contractions)   (PSUM fp32 accum, bias fused in ACT)
        oT = w2-contractions over hT
    with D/H features on the partition axis end-to-end, so no on-device
    transposes are needed anywhere.
  - Host applies the combine weights and scatter-adds per-expert outputs back
    into the full [B,S,D] output (each token appears in exactly 2 experts).

v3 schedule notes (from trace analysis):
  - The ramp is HBM-bandwidth-bound: the PE cannot start until xb0+w1[0]
    arrive (~2.6us of transfer after the ~8.3us fixed prologue), and the
    first `lead` groups consume slab bandwidth at ~200GB/s.  v1 lost 8.3us
    because xblk2 and a hoisted 1MB w2[0] fetch jumped ahead of w1 slabs in
    the queues.  Queue FIFO order is now explicit: xb0 halves first (one per
    queue), then w1 slabs alternating, with xblk1/xblk2 inserted after
    w1[4]/w1[5].
  - w2 fetches are hard-gated behind layer-1 progress: the scheduler hoists
    any ungated DMA descriptor to the top of the stream, so each early w2
    tile's buffer is first touched by a tiny scalar memset placed (in scalar
    program order) after a mid-L1 activation; the WAR dependency delays the
    fetch until ~30us, long before L2 needs it at ~125us.
  - C pads to /8 (1072 here) instead of /32 (1088): matmul column counts and
    DMA row lengths have no /32 requirement.  Saves 3.4us of matmul time.
  - 2 DMA queues only (sync+scalar); fewer queues/semaphores also trim the
    fixed teardown epilogue.

Per-core layouts (E=8 experts, D=1024, H=4096, C tokens):
  xT  [128, 8*C]         bf16   block-major concat of xT[p, ko, t] = x_g[t, ko*128+p]
  w1t [128, 32, 8, 128]  bf16   w1t[p, ho, ko, j]  = w1[e][ho*128+j, ko*128+p]
  w2t [128, 8, 32, 128]  bf16   w2t[p, do, ko, j]  = w2[e][do*128+j, ko*128+p]
  b1t [128, 32]          f32    b1t[p, ho]         = b1[e][ho*128+p]
  oT  [128, 8, C]        f32    oT[p, do, t]       = o_g[t, do*128+p]
"""

import numpy as np
import ml_dtypes

TOP_K = 2
P = 128
D = 1024
H = 4096
E = 8

_COMPILED = {}  # C -> compiled Bacc instance


def _token_blocks(C):
    """First block 320 (fast start; slab demand still below delivery),
    remaining blocks <=512, all >=232 so LDWEIGHTS stays hidden."""
    if C <= 512:
        return [C]
    b0 = 320
    rem = C - b0
    nblk = -(-rem // 480)
    sizes = [b0]
    for i in range(nblk):
        s = -(-rem // (nblk - i))
        s = ((s + 7) // 8) * 8
        s = min(s, rem)
        sizes.append(s)
        rem -= s
    assert sum(sizes) == C and all(s <= 512 for s in sizes), sizes
    return sizes


def _build_ffn_kernel(C):
    import concourse.mybir as mybir
    import concourse.tile as tile
    from concourse import bacc

    blocks = _token_blocks(C)
    starts = [sum(blocks[:i]) for i in range(len(blocks))]
    NTMAX = max(blocks)
    nb = len(blocks)
    bf16 = mybir.dt.bfloat16
    f32 = mybir.dt.float32

    nc = bacc.Bacc("TRN2", target_bir_lowering=False, debug=False)
    xT = nc.dram_tensor("xT", [P, D // P * C], bf16, kind="ExternalInput").ap()
    w1t = nc.dram_tensor("w1t", [P, H // P, D // P, P], bf16, kind="ExternalInput").ap()
    w2t = nc.dram_tensor("w2t", [P, D // P, H // P, P], bf16, kind="ExternalInput").ap()
    b1t = nc.dram_tensor("b1t", [P, H // P], f32, kind="ExternalInput").ap()
    oT = nc.dram_tensor("oT", [P, D // P, C], f32, kind="ExternalOutput").ap()

    KH = D // P // 2  # 4: ko half-count for the split first block / slab

    with tile.TileContext(nc) as tc:
        with (
            tc.tile_pool(name="const", bufs=1) as cpool,
            tc.tile_pool(name="resident", bufs=1) as rpool,
            tc.tile_pool(name="warm", bufs=1) as warmpool,
            tc.tile_pool(name="w1p", bufs=6) as w1pool,
            tc.tile_pool(name="w2p", bufs=3) as w2pool,
            tc.tile_pool(name="ost", bufs=4) as opool,
            tc.tile_pool(name="ps", bufs=4, space="PSUM") as pspool,
            tc.tile_pool(name="wps", bufs=1, space="PSUM") as wpspool,
        ):
            # PE warm-up: dependency-free matmuls keep the HAM clock-gate at
            # 8/8 while the first input DMAs are in flight.
            wsrc = warmpool.tile([P, 512], bf16)
            nc.gpsimd.memset(wsrc[:], 0.0)
            wps = wpspool.tile([P, 512], f32)
            for _ in range(13):
                nc.tensor.matmul(wps[:], wsrc[:, :P], wsrc[:], start=True, stop=True)

            # ---- Input DMA schedule (FIFO order per queue IS the schedule):
            # qA (sync):   w1 pairs (0,1), (2,3), ... — a pure slab stream
            #              with 4096B rows (2048B rows proved ~2-3x slower
            #              per queue in the trace).
            # qB (scalar): xb0, xblk1, xblk2 (big rows), then gated w2.
            # qC (gpsimd): b1 (tiny).
            sz0 = blocks[0]
            b1_sb = cpool.tile([P, H // P], f32)
            nc.gpsimd.dma_start(b1_sb[:], b1t[:])

            w1_tiles = {}

            def fetch_w1(ho):
                # fetch the aligned pair (ho&~1, ho|1) as one 4096B-row DMA
                base = ho & ~1
                t = w1pool.tile([P, 2, D // P, P], bf16, tag="w1s")
                nc.sync.dma_start(t[:], w1t[:, base : base + 2])
                w1_tiles[base] = t
                w1_tiles[base + 1] = t

            fetch_w1(0)
            x_blks = []
            for blk in range(nb):
                st, sz = starts[blk], blocks[blk]
                xb = rpool.tile([P, D // P * sz], bf16, tag=f"xb{blk}")
                nc.scalar.dma_start(xb[:], xT[:, D // P * st : D // P * (st + sz)])
                x_blks.append(xb)

            h_sb = rpool.tile([P, H // P, C], bf16)

            def w1_src(ho, ko):
                return w1_tiles[ho][:, ho % 2, ko, :]

            def x_src(blk, ko):
                sz = blocks[blk]
                return x_blks[blk][:, ko * sz : (ko + 1) * sz]

            # Group order: lead = first 6 ho rows on block 0 (x blocks 1/2
            # still in flight), then their remaining blocks, then ho-major
            # triples.  Keeps slab lifetimes short (pool of 10 suffices) and
            # slab demand low after the ramp.
            lead = min(6, H // P) if nb >= 2 else 0
            pairs = [(k, 0) for k in range(lead)]
            for k in range(lead):
                pairs += [(k, b) for b in range(1, nb)]
            for ho in range(lead, H // P):
                pairs += [(ho, b) for b in range(nb)]

            # Layer 1: hT[:, ho, t] = gelu(sum_ko w1t[:,ho,ko,:].T @ x + b1)
            w2_early = []
            for pi, (ho, blk) in enumerate(pairs):
                if ho not in w1_tiles:
                    fetch_w1(ho)
                st, sz = starts[blk], blocks[blk]
                ps = pspool.tile([P, NTMAX], f32, tag="ps")
                for ko in range(D // P):
                    nc.tensor.matmul(
                        ps[:, :sz],
                        w1_src(ho, ko),
                        x_src(blk, ko),
                        start=(ko == 0),
                        stop=(ko == D // P - 1),
                    )
                nc.scalar.activation(
                    h_sb[:, ho, st : st + sz],
                    ps[:, :sz],
                    mybir.ActivationFunctionType.Gelu,
                    bias=b1_sb[:, ho : ho + 1],
                )
                if pi == 16:
                    # Early w2 fetches, gated so they cannot hoist ahead of
                    # the ramp: each buffer is touched by a scalar memset
                    # that (scalar is in-order) runs after the ACT above.
                    for do in range(min(3, D // P)):
                        w2s = w2pool.tile([P, H // P, P], bf16, tag="w2s")
                        nc.scalar.memzero(w2s[:, 0:2])
                        eng = nc.sync if do % 2 == 0 else nc.scalar
                        eng.dma_start(w2s[:], w2t[:, do])
                        w2_early.append(w2s)

            # Layer 2: oT[:, do, t] = sum_ko w2t[:,do,ko,:].T @ hT[:,ko,t]
            for do in range(D // P):
                if do < len(w2_early):
                    w2s = w2_early[do]
                else:
                    w2s = w2pool.tile([P, H // P, P], bf16, tag="w2s")
                    eng = nc.sync if do % 2 == 0 else nc.scalar
                    eng.dma_start(w2s[:], w2t[:, do])
                for bi, (st, sz) in enumerate(zip(starts, blocks)):
                    ps = pspool.tile([P, NTMAX], f32, tag="ps")
                    for ko in range(H // P):
                        nc.tensor.matmul(
                            ps[:, :sz],
                            w2s[:, ko, :],
                            h_sb[:, ko, st : st + sz],
                            start=(ko == 0),
                            stop=(ko == H // P - 1),
                        )
                    last = do == D // P - 1 and bi == nb - 1
                    if not last:
                        ob = opool.tile([P, NTMAX], f32, tag="ob")
                        nc.vector.tensor_copy(ob[:, :sz], ps[:, :sz])
                        nc.scalar.dma_start(oT[:, do, st : st + sz], ob[:, :sz])
                    else:
                        # Final eviction is on the critical path: split it so
                        # the first half's DMA overlaps the second half's
                        # copy, using both queues.
                        hsz = sz // 2
                        ob = opool.tile([P, NTMAX], f32, tag="ob")
                        nc.vector.tensor_copy(ob[:, :hsz], ps[:, :hsz])
                        nc.sync.dma_start(oT[:, do, st : st + hsz], ob[:, :hsz])
                        nc.vector.tensor_copy(ob[:, hsz:sz], ps[:, hsz:sz])
                        nc.scalar.dma_start(
                            oT[:, do, st + hsz : st + sz], ob[:, hsz:sz]
                        )

    nc.compile()
    return nc


def _route_host(x_flat, router_w):
    """Float64 router: returns per-expert (token_idx, combine_weight)."""
    logits = x_flat.astype(np.float64) @ router_w.astype(np.float64).T
    m = logits.max(axis=-1, keepdims=True)
    p = np.exp(logits - m)
    p /= p.sum(axis=-1, keepdims=True)
    order = np.argsort(-p, axis=-1)
    topi = order[:, :TOP_K]
    topw = np.take_along_axis(p, topi, axis=-1)
    topw /= topw.sum(axis=-1, keepdims=True)

    idx_list, wgt_list = [], []
    for e in range(E):
        mask = topi == e  # [T, TOP_K]; at most one True per row
        rows = np.nonzero(mask.any(axis=-1))[0]
        w = topw[rows][mask[rows]]
        idx_list.append(rows)
        wgt_list.append(w.astype(np.float32))
    return idx_list, wgt_list


def kernel(x, router_w, w1, b1, w2, b2):
    from concourse import bass_utils

    x = np.asarray(x)
    router_w = np.asarray(router_w)
    w1 = np.asarray(w1)
    b1 = np.asarray(b1)
    w2 = np.asarray(w2)
    b2 = np.asarray(b2)

    B, S, _ = x.shape
    T = B * S
    x_flat = x.reshape(T, D)

    idx_list, wgt_list = _route_host(x_flat, router_w)
    max_cnt = max(len(i) for i in idx_list)
    C = ((max_cnt + 7) // 8) * 8

    if C not in _COMPILED:
        _COMPILED[C] = _build_ffn_kernel(C)
    nc = _COMPILED[C]

    blocks = _token_blocks(C)
    starts = [sum(blocks[:i]) for i in range(len(blocks))]
    bf = ml_dtypes.bfloat16
    in_maps = []
    for e in range(E):
        idx = idx_list[e]
        n_e = len(idx)
        # xT [128, 8*C] block-major: pad tokens to C with zeros
        xg = np.zeros((C, D), np.float32)
        xg[:n_e] = x_flat[idx]
        xT_full = xg.T.reshape(D // P, P, C).transpose(1, 0, 2)  # [128, 8, C]
        xT_d = np.concatenate(
            [xT_full[:, :, st : st + sz].reshape(P, -1) for st, sz in zip(starts, blocks)],
            axis=1,
        ).astype(bf)
        w1_d = np.ascontiguousarray(
            w1[e].reshape(H // P, P, D // P, P).transpose(3, 0, 2, 1)
        ).astype(bf)
        w2_d = np.ascontiguousarray(
            w2[e].reshape(D // P, P, H // P, P).transpose(3, 0, 2, 1)
        ).astype(bf)
        b1_d = np.ascontiguousarray(b1[e].reshape(H // P, P).T).astype(np.float32)
        in_maps.append({"xT": xT_d, "w1t": w1_d, "w2t": w2_d, "b1t": b1_d})

    res = bass_utils.run_bass_kernel_spmd(nc, in_maps, core_ids=list(range(E)))

    out = np.zeros((T, D), np.float32)
    for e in range(E):
        idx = idx_list[e]
        n_e = len(idx)
        oT = res.results[e]["oT"]  # [128, 8, C]
        o_g = oT.transpose(1, 0, 2).reshape(D, C)[:, :n_e].T  # [n_e, D]
        out[idx] += wgt_list[e][:, None] * (o_g + b2[e][None, :])
    return out.reshape(B, S, D).astype(np.float32)


# revision 16
# speedup vs baseline: 1.0289x; 1.0187x over previous
"""MoE top-2 routing kernel for 8 Trainium2 NeuronCores.

Strategy (expert-parallel with two-segment load balancing):
  - Host computes the (tiny) router in float64: logits -> softmax -> top-2 ->
    renormalize.  Selection was verified tie-safe: min prob gap between
    2nd/3rd expert is ~8e-6 while cross-backend fp32 logit noise is ~3e-7.
  - Plain expert-parallel pads every core to the max expert count (1072 for
    this input vs 1024 average).  Instead each core gets TWO fixed-size
    segments (a, b), each with its own expert weight set; the host assigns
    expert -> slot multisets: the 2 largest experts take two a-slots
    (2a >= c_max), the 2 smallest take two b-slots, the middle four take
    one a + one b (a+b >= c_mid).  For this input a=536, b=504: capacity
    1040/core instead of 1072 (-6.9us of matmul).  Cores stream 2 full
    weight sets (33.6MB, ~150GB/s — well under the ~310GB/s queue budget),
    and all token blocks stay >=232 columns so LDWEIGHTS stays hidden.
  - Each core runs a dense FFN per segment:
        hT = gelu(w1T.T-contractions)   (PSUM fp32 accum, bias fused in ACT)
        oT = w2-contractions over hT
    with D/H features on the partition axis end-to-end (no on-device
    transposes).  Host applies combine weights and scatter-adds outputs.

Schedule notes (from perfetto/ntff trace analysis):
  - The ramp is HBM-bandwidth-bound.  Queue FIFO order IS the schedule:
    sync carries a pure w1 stream in ho-PAIRS (4096B DMA rows; 2048B rows
    measured 2-3x slower per queue), scalar carries the x blocks, gpsimd
    carries b1.
  - w2 fetches are hard-gated behind layer-1 progress: the scheduler hoists
    any ungated DMA descriptor to the top of the stream, so the early w2
    buffers are first touched by a tiny scalar memzero placed (scalar is
    in-order) after a mid-L1 activation; the WAR dep delays the fetch until
    ~35us, long before L2 needs it.
  - 13 dependency-free warm-up matmuls bridge the ~8.3us fixed prologue so
    the HAM clock-gate is at 8/8 when real matmuls start; steady state runs
    at the bf16 peak (2 cols/cycle).
  - Remaining fixed costs (framework): ~8.3us prologue before the first DMA
    packet, ~8.8us BSP semaphore-teardown epilogue.

Per-core layouts (D=1024, H=4096; cap = a+b tokens, A span [0,a), B [a,cap)):
  xT   [128, 8*cap]        bf16   block-major: xT[p, ko, t] = x_g[t, ko*128+p]
  w1A/B [128, 32, 8, 128]  bf16   w1s[p, ho, ko, j] = w1[e][ho*128+j, ko*128+p]
  w2A/B [128, 8, 32, 128]  bf16   w2s[p, do, ko, j] = w2[e][do*128+j, ko*128+p]
  b1A/B [128, 32]          f32    b1s[p, ho]        = b1[e][ho*128+p]
  oT   [128, 8, cap]       f32    oT[p, do, t]      = o_g[t, do*128+p]
"""

import numpy as np
import ml_dtypes

TOP_K = 2
P = 128
D = 1024
H = 4096
E = 8

_COMPILED = {}  # (a, b) or ('single', C) -> compiled Bacc instance


def _ceil8(n):
    return ((n + 7) // 8) * 8


def _split_seg(S, first):
    """Split segment of S tokens into blocks: `first`-sized lead block, the
    rest as even blocks <=512 (all >=232 when S allows, so LDWEIGHTS stays
    hidden behind the previous matmul)."""
    if S <= 512:
        return [S]
    b0 = min(first, S - 232)
    rem = S - b0
    nblk = -(-rem // 480)
    sizes = [b0]
    for i in range(nblk):
        s = -(-rem // (nblk - i))
        s = min(_ceil8(s), rem)
        sizes.append(s)
        rem -= s
    assert sum(sizes) == S and all(s <= 512 for s in sizes), sizes
    return sizes


def _seg_blocks(a, b):
    return _split_seg(a, 304), _split_seg(b, 272)


def _build_dual_kernel(a, b):
    import concourse.mybir as mybir
    import concourse.tile as tile
    from concourse import bacc

    blocks_a, blocks_b = _seg_blocks(a, b)
    blocks = blocks_a + blocks_b
    seg_of = [0] * len(blocks_a) + [1] * len(blocks_b)
    starts = [sum(blocks[:i]) for i in range(len(blocks))]
    cap = a + b
    NTMAX = max(blocks)
    nb = len(blocks)
    nba = len(blocks_a)
    bf16 = mybir.dt.bfloat16
    f32 = mybir.dt.float32

    nc = bacc.Bacc("TRN2", target_bir_lowering=False, debug=False)
    xT = nc.dram_tensor("xT", [P, D // P * cap], bf16, kind="ExternalInput").ap()
    w1d = [
        nc.dram_tensor(n, [P, H // P, D // P, P], bf16, kind="ExternalInput").ap()
        for n in ("w1A", "w1B")
    ]
    w2d = [
        nc.dram_tensor(n, [P, D // P, H // P, P], bf16, kind="ExternalInput").ap()
        for n in ("w2A", "w2B")
    ]
    b1d = [
        nc.dram_tensor(n, [P, H // P], f32, kind="ExternalInput").ap()
        for n in ("b1A", "b1B")
    ]
    oT = nc.dram_tensor("oT", [P, D // P, cap], f32, kind="ExternalOutput").ap()

    with tile.TileContext(nc) as tc:
        with (
            tc.tile_pool(name="const", bufs=1) as cpool,
            tc.tile_pool(name="resident", bufs=1) as rpool,
            tc.tile_pool(name="warm", bufs=1) as warmpool,
            tc.tile_pool(name="w1p", bufs=6) as w1pool,
            tc.tile_pool(name="w2p", bufs=3) as w2pool,
            tc.tile_pool(name="ost", bufs=4) as opool,
            tc.tile_pool(name="ps", bufs=4, space="PSUM") as pspool,
            tc.tile_pool(name="wps", bufs=1, space="PSUM") as wpspool,
        ):
            # PE warm-up: dependency-free matmuls keep the HAM clock-gate at
            # 8/8 while the first input DMAs are in flight.
            wsrc = warmpool.tile([P, 512], bf16)
            nc.gpsimd.memset(wsrc[:], 0.0)
            wps = wpspool.tile([P, 512], f32)
            for _ in range(13):
                nc.tensor.matmul(wps[:], wsrc[:, :P], wsrc[:], start=True, stop=True)

            b1A_sb = cpool.tile([P, H // P], f32, tag="b1A")
            b1B_sb = cpool.tile([P, H // P], f32, tag="b1B")
            b1_sb = [b1A_sb, b1B_sb]
            nc.gpsimd.dma_start(b1A_sb[:], b1d[0][:])
            nc.gpsimd.dma_start(b1B_sb[:], b1d[1][:])

            w1_tiles = {}

            def fetch_w1(seg, ho):
                base = ho & ~1
                t = w1pool.tile([P, 2, D // P, P], bf16, tag="w1s")
                nc.sync.dma_start(t[:], w1d[seg][:, base : base + 2])
                w1_tiles[(seg, base)] = t
                w1_tiles[(seg, base + 1)] = t

            fetch_w1(0, 0)
            x_blks = []
            for blk in range(nb):
                st, sz = starts[blk], blocks[blk]
                xb = rpool.tile([P, D // P * sz], bf16, tag=f"xb{blk}")
                nc.scalar.dma_start(xb[:], xT[:, D // P * st : D // P * (st + sz)])
                x_blks.append(xb)

            h_sb = rpool.tile([P, H // P, cap], bf16)

            def w1_src(seg, ho, ko):
                return w1_tiles[(seg, ho)][:, ho % 2, ko, :]

            def x_src(blk, ko):
                sz = blocks[blk]
                return x_blks[blk][:, ko * sz : (ko + 1) * sz]

            # Group order: lead = first 6 ho rows on block 0 (later x blocks
            # still in flight), then their remaining A blocks, then ho-major
            # over segment A, then ho-major over segment B (its w1 stream
            # arrives during A's compute).  Keeps slab lifetimes short and
            # slab demand well under delivery after the ramp.
            lead = min(6, H // P) if nba >= 2 else 0
            pairs = [(k, 0) for k in range(lead)]
            for k in range(lead):
                pairs += [(k, bi) for bi in range(1, nba)]
            for ho in range(lead, H // P):
                pairs += [(ho, bi) for bi in range(nba)]
            for ho in range(H // P):
                pairs += [(ho, bi) for bi in range(nba, nb)]

            # Layer 1: hT[:, ho, t] = gelu(sum_ko w1.T @ x + b1)
            w2_early = []
            for pi, (ho, blk) in enumerate(pairs):
                seg = seg_of[blk]
                if (seg, ho) not in w1_tiles:
                    fetch_w1(seg, ho)
                st, sz = starts[blk], blocks[blk]
                ps = pspool.tile([P, NTMAX], f32, tag="ps")
                for ko in range(D // P):
                    nc.tensor.matmul(
                        ps[:, :sz],
                        w1_src(seg, ho, ko),
                        x_src(blk, ko),
                        start=(ko == 0),
                        stop=(ko == D // P - 1),
                    )
                nc.scalar.activation(
                    h_sb[:, ho, st : st + sz],
                    ps[:, :sz],
                    mybir.ActivationFunctionType.Gelu,
                    bias=b1_sb[seg][:, ho : ho + 1],
                )
                if pi == 16:
                    # Early w2A fetches, gated so they cannot hoist ahead of
                    # the ramp: each buffer is touched by a scalar memzero
                    # that (scalar is in-order) runs after the ACT above.
                    for do in range(min(3, D // P)):
                        w2s = w2pool.tile([P, H // P, P], bf16, tag="w2s")
                        nc.scalar.memzero(w2s[:, 0:2])
                        eng = nc.sync if do % 2 == 0 else nc.scalar
                        eng.dma_start(w2s[:], w2d[0][:, do])
                        w2_early.append(w2s)

            # Layer 2: oT[:, do, t] = sum_ko w2[:,do,ko,:].T @ hT[:,ko,t]
            for seg in range(2):
                sblks = [i for i in range(nb) if seg_of[i] == seg]
                for do in range(D // P):
                    if seg == 0 and do < len(w2_early):
                        w2s = w2_early[do]
                    else:
                        w2s = w2pool.tile([P, H // P, P], bf16, tag="w2s")
                        eng = nc.sync if do % 2 == 0 else nc.scalar
                        eng.dma_start(w2s[:], w2d[seg][:, do])
                    for bi in sblks:
                        st, sz = starts[bi], blocks[bi]
                        ps = pspool.tile([P, NTMAX], f32, tag="ps")
                        for ko in range(H // P):
                            nc.tensor.matmul(
                                ps[:, :sz],
                                w2s[:, ko, :],
                                h_sb[:, ko, st : st + sz],
                                start=(ko == 0),
                                stop=(ko == H // P - 1),
                            )
                        last = seg == 1 and do == D // P - 1 and bi == nb - 1
                        if not last:
                            ob = opool.tile([P, NTMAX], f32, tag="ob")
                            nc.vector.tensor_copy(ob[:, :sz], ps[:, :sz])
                            nc.scalar.dma_start(oT[:, do, st : st + sz], ob[:, :sz])
                        else:
                            # Final eviction is on the critical path: split it
                            # so the first half's DMA overlaps the second
                            # half's copy, using both queues.
                            hsz = sz // 2
                            ob = opool.tile([P, NTMAX], f32, tag="ob")
                            nc.vector.tensor_copy(ob[:, :hsz], ps[:, :hsz])
                            nc.sync.dma_start(oT[:, do, st : st + hsz], ob[:, :hsz])
                            nc.vector.tensor_copy(ob[:, hsz:sz], ps[:, hsz:sz])
                            nc.scalar.dma_start(
                                oT[:, do, st + hsz : st + sz], ob[:, hsz:sz]
                            )

    nc.compile()
    return nc


def _route_host(x_flat, router_w):
    """Float64 router: returns per-expert (token_idx, combine_weight)."""
    logits = x_flat.astype(np.float64) @ router_w.astype(np.float64).T
    m = logits.max(axis=-1, keepdims=True)
    p = np.exp(logits - m)
    p /= p.sum(axis=-1, keepdims=True)
    order = np.argsort(-p, axis=-1)
    topi = order[:, :TOP_K]
    topw = np.take_along_axis(p, topi, axis=-1)
    topw /= topw.sum(axis=-1, keepdims=True)

    idx_list, wgt_list = [], []
    for e in range(E):
        mask = topi == e  # [T, TOP_K]; at most one True per row
        rows = np.nonzero(mask.any(axis=-1))[0]
        w = topw[rows][mask[rows]]
        idx_list.append(rows)
        wgt_list.append(w.astype(np.float32))
    return idx_list, wgt_list


def _plan_slots(counts):
    """Two-segment balancing: returns (a, b, slots) where slots is a list of
    8 (expert_a, expert_b) core assignments, or None if not profitable.
    Each expert's tokens are later split greedily across its slots."""
    order = sorted(range(E), key=lambda e: -counts[e])
    big, mid, small = order[:2], order[2:-2], order[-2:]
    a = _ceil8(-(-max(counts[e] for e in big) // 2))
    b = _ceil8(-(-max(counts[e] for e in small) // 2))
    need_mid = max(counts[e] for e in mid)
    if a + b < need_mid:
        b = _ceil8(need_mid - a)
    # feasibility + profitability vs single-segment
    if 2 * b < max(counts[e] for e in small) or a < 466 or b < 466:
        return None  # segments must each split into >=232-col blocks
    if a + b >= _ceil8(max(counts)):
        return None
    slots_a = [big[0], big[0], big[1], big[1]] + mid
    slots_b = [small[0], small[0], small[1], small[1]] + mid
    return a, b, list(zip(slots_a, slots_b))


def kernel(x, router_w, w1, b1, w2, b2):
    from concourse import bass_utils

    x = np.asarray(x)
    router_w = np.asarray(router_w)
    w1 = np.asarray(w1)
    b1 = np.asarray(b1)
    w2 = np.asarray(w2)
    b2 = np.asarray(b2)

    B, S, _ = x.shape
    T = B * S
    x_flat = x.reshape(T, D)

    idx_list, wgt_list = _route_host(x_flat, router_w)
    counts = [len(i) for i in idx_list]
    plan = _plan_slots(counts)
    if plan is None:
        # degenerate fallback: every core hosts its own expert in both
        # segments (capacity = single-segment capacity, still correct)
        cmax = max(counts)
        a = _ceil8(-(-cmax // 2))
        b = _ceil8(cmax - a)
        plan = (a, b, [(e, e) for e in range(E)])
    a, b, slots = plan

    key = (a, b)
    if key not in _COMPILED:
        _COMPILED[key] = _build_dual_kernel(a, b)
    nc = _COMPILED[key]

    blocks_a, blocks_b = _seg_blocks(a, b)
    blocks = blocks_a + blocks_b
    starts = [sum(blocks[:i]) for i in range(len(blocks))]
    cap = a + b
    bf = ml_dtypes.bfloat16

    # split each expert's tokens greedily across its slots (a-slots first)
    seg_size = {0: a, 1: b}
    slot_tokens = [[None, None] for _ in range(E)]  # per core: [A idx, B idx]
    slot_wgts = [[None, None] for _ in range(E)]
    used = {e: 0 for e in range(E)}
    for seg in range(2):
        for c in range(E):
            e = slots[c][seg]
            s = seg_size[seg]
            lo = used[e]
            hi = min(lo + s, counts[e])
            used[e] = hi
            slot_tokens[c][seg] = idx_list[e][lo:hi]
            slot_wgts[c][seg] = wgt_list[e][lo:hi]
    for e in range(E):
        assert used[e] == counts[e], (e, used[e], counts[e])

    # pre-transpose each expert's weights once; slots share the arrays
    w1_d, w2_d, b1_d = {}, {}, {}
    for e in set(s for pair in slots for s in pair):
        w1_d[e] = np.ascontiguousarray(
            w1[e].reshape(H // P, P, D // P, P).transpose(3, 0, 2, 1)
        ).astype(bf)
        w2_d[e] = np.ascontiguousarray(
            w2[e].reshape(D // P, P, H // P, P).transpose(3, 0, 2, 1)
        ).astype(bf)
        b1_d[e] = np.ascontiguousarray(b1[e].reshape(H // P, P).T).astype(np.float32)

    in_maps = []
    for c in range(E):
        ea, eb = slots[c]
        xg = np.zeros((cap, D), np.float32)
        na = len(slot_tokens[c][0])
        nb_ = len(slot_tokens[c][1])
        xg[:na] = x_flat[slot_tokens[c][0]]
        xg[a : a + nb_] = x_flat[slot_tokens[c][1]]
        xT_full = xg.T.reshape(D // P, P, cap).transpose(1, 0, 2)  # [128, 8, cap]
        xT_d = np.concatenate(
            [xT_full[:, :, st : st + sz].reshape(P, -1) for st, sz in zip(starts, blocks)],
            axis=1,
        ).astype(bf)
        in_maps.append(
            {
                "xT": xT_d,
                "w1A": w1_d[ea], "w2A": w2_d[ea], "b1A": b1_d[ea],
                "w1B": w1_d[eb], "w2B": w2_d[eb], "b1B": b1_d[eb],
            }
        )

    res = bass_utils.run_bass_kernel_spmd(nc, in_maps, core_ids=list(range(E)))

    out = np.zeros((T, D), np.float32)
    for c in range(E):
        ea, eb = slots[c]
        oT = res.results[c]["oT"]  # [128, 8, cap]
        o_g = oT.transpose(1, 0, 2).reshape(D, cap).T  # [cap, D]
        na = len(slot_tokens[c][0])
        nb_ = len(slot_tokens[c][1])
        if na:
            out[slot_tokens[c][0]] += slot_wgts[c][0][:, None] * (
                o_g[:na] + b2[ea][None, :]
            )
        if nb_:
            out[slot_tokens[c][1]] += slot_wgts[c][1][:, None] * (
                o_g[a : a + nb_] + b2[eb][None, :]
            )
    return out.reshape(B, S, D).astype(np.float32)
